# revision 19
# baseline (speedup 1.0000x reference)
"""Trainium2 Bass kernel: conv2d(3x3,VALID) + bias -> min over C_out -> tanh(tanh).

Full-input contract: kernel(**inputs) takes the unsharded inputs
  x:           [32, 16, 256, 256] f32
  conv_weight: [64, 16, 3, 3]     f32
  conv_bias:   [64]               f32
and returns [32, 1, 254, 254] f32.

Strategy (data-parallel over batch, 4 images per core on 8 cores):
The conv is cast as matmuls over a "kw-shifted slab" in SBUF:
  slab[kw*16+c, t] = x[c, t+kw]   (kw in 0..6, flattened image cols t)
plus a ones row (partition 112) that carries the bias through the matmul.
For a block of 640 flat positions p = base + 5*m + j (m in 0..127, j in 0..4):
  out[m, (j,o)] = sum_kh  slab[:, base+kh*256+5m].T @ wmov[kh]
with wmov[kh][kw*16+c, j*64+o] = W[o,c,kh,kw-j] (0 when kw-j not in 0..2).
This yields PSUM [128 positions, 5 shifts, 64 channels]; the channel-min is
then a free-dim reduce_min on DVE, followed by tanh(tanh()) on ACT.
Computed on the full 256-wide rows; the host drops the 2 garbage columns/rows.

The slab is float8_e3m4: 1 byte/elem halves the HBM traffic that bounds the
fp16 version (~175 GB/s/core effective), and 4 mantissa bits keep the
x-quantization noise at ~1.6e-2 final rel err (vs the 2e-2 gate; e4m3's 3
mantissa bits measure 4.5e-2 — too coarse). The weights stay fp16 (they are
only ~217 KB, their dtype doesn't touch the DMA roofline) and the PE runs
mixed-dtype f8e3 x f16 matmuls, which TRN2 supports.

Outputs are buffered per window in SBUF [128, 51, 5] f16 and stored with one
SWDGE DMA of 128 contiguous 510-elem partition lines; the host de-permutes.
(Storing per-chunk in position order needs 8-byte descriptors — measured
~25us/engine of pure descriptor overhead across the 16 SDMA engines.)
"""

import sys
import types

import numpy as np
import ml_dtypes

# ---------------------------------------------------------------------------
# NTFF profile hook registration (the container's antenv stub lacks
# axon_hooks; registering it enables trace=True for profiling runs).
def _install_axon_hooks():
    try:
        import antenv.axon_hooks  # noqa: F401
        return
    except ImportError:
        pass
    try:
        import antenv
        from trn_agent_boot.trn_boot import _ntff_profile_via_ctypes
    except ImportError:
        return
    mod = types.ModuleType("antenv.axon_hooks")
    _hook = [None]
    mod.set_axon_ntff_profile_hook = lambda h: _hook.__setitem__(0, h)
    mod.get_axon_ntff_profile_hook = lambda: _hook[0]
    sys.modules["antenv.axon_hooks"] = mod
    antenv.axon_hooks = mod
    try:
        mod.set_axon_ntff_profile_hook(
            _ntff_profile_via_ctypes("/opt/axon/libaxon_pjrt.so")
        )
    except Exception:
        pass


_install_axon_hooks()

import concourse.bass as bass  # noqa: E402
import concourse.tile as tile  # noqa: E402
from concourse import bacc, mybir  # noqa: E402
from concourse.bass_utils import run_bass_kernel_spmd  # noqa: E402

N_CORES = 8
IMGS_PER_CORE = 4
C_IN, H, W = 16, 256, 256
C_OUT = 64
OH = OW = 254

J = 5                 # position shifts per matmul column group
WK = 7                # kw taps present in the contraction (0..6)
KDIM = WK * C_IN + 1  # 113 partitions: 7 kw-shifts x 16 ch + ones row
KD0 = WK * C_IN       # 112 data partitions
NFREE = J * C_OUT     # 320 moving columns
BLK = 128 * J         # 640 flat positions per block
IMG = H * W           # 65536
PAD_COLS = 66560      # 260 rows of 256 (2 extra conv rows + slack)
NBLOCKS = 102         # blocks 0..101 cover flat positions 0..65279

WIN_BLOCKS = 51       # two slab windows per image
WIN_STRIDE = WIN_BLOCKS * BLK      # 32640
WIN_COLS = 33792                   # window cols; max read rel offset 33153
CHUNKS = [8, 8, 8, 8, 8, 8, 3]     # blocks per output chunk inside a window


def _prep_inputs(x, conv_weight, conv_bias):
    """Host-side packing: slab-layout fp8 image tensor and fp16 weights.

    x7[i, kw*16+c, t] = x[i, c, t+kw] (kw in 0..6), row 112 = ones — exactly
    the SBUF slab layout, so device loads are single contiguous DMAs.
    """
    f8 = ml_dtypes.float8_e3m4
    n = x.shape[0]
    xf = x.reshape(n, C_IN, IMG).astype(f8)
    x7 = np.zeros((n, KDIM, PAD_COLS), dtype=f8)
    for kw in range(WK):
        lo = max(0, IMG - kw)
        x7[:, kw * C_IN:(kw + 1) * C_IN, :lo] = xf[:, :, kw:kw + lo]
    x7[:, KD0, :] = 1.0

    # wmov[kh][kw*16+c, j*64+o] = W[o, c, kh, kw-j] for kw-j in 0..2
    wmov = np.zeros((KDIM, 3, J, C_OUT), dtype=np.float32)
    for kh in range(3):
        for kw in range(WK):
            for j in range(J):
                kk = kw - j
                if 0 <= kk <= 2:
                    wmov[kw * C_IN:(kw + 1) * C_IN, kh, j, :] = (
                        conv_weight[:, :, kh, kk].T
                    )
    wmov[KD0, 0, :, :] = conv_bias[None, :]  # bias via ones row, kh=0 only
    # dram layout [113, 3*320] with col = kh*320 + j*64 + o
    wmov = wmov.reshape(KDIM, 3 * NFREE).astype(np.float16)
    return x7, wmov


def _build_program():
    nc = bacc.Bacc(
        "TRN2", target_bir_lowering=False, debug=False, num_devices=N_CORES
    )
    f8 = mybir.dt.float8e3
    f16 = mybir.dt.float16
    f32 = mybir.dt.float32

    x_d = nc.dram_tensor(
        "x", [IMGS_PER_CORE, KDIM, PAD_COLS], f8, kind="ExternalInput"
    )
    w_d = nc.dram_tensor("w", [KDIM, 3 * NFREE], f16, kind="ExternalInput")
    # y layout [img, window, m, window-block, j]; host de-permutes
    y_d = nc.dram_tensor(
        "y", [IMGS_PER_CORE, 2, 128, WIN_BLOCKS, J], f16,
        kind="ExternalOutput",
    )

    with tile.TileContext(nc) as tc:
        with (
            tc.tile_pool(name="wpool", bufs=1) as wpool,
            tc.tile_pool(name="slab", bufs=3) as slab_pool,
            tc.tile_pool(name="stage", bufs=4) as stage_pool,
            tc.tile_pool(name="thwin", bufs=2) as th_pool,
            tc.tile_pool(name="psum", bufs=2, space="PSUM") as psum_pool,
        ):
            w_t = wpool.tile([KDIM, 3 * NFREE], f16)
            # Split the weight load: a 113-partition transfer lands on a
            # single DMA engine (~16us serial at startup); 112+1 sprays.
            nc.sync.dma_start(w_t[0:KD0, :], w_d[0:KD0, :])
            nc.sync.dma_start(w_t[KD0:KDIM, :], w_d[KD0:KDIM, :])

            windows = [
                (i, wi) for i in range(IMGS_PER_CORE) for wi in range(2)
            ]

            def load_window(idx, parts):
                i, wi = windows[idx]
                wstart = wi * WIN_STRIDE
                slab = slab_pool.tile([KDIM, WIN_COLS], f8)
                # Ones row first (small, separate so the data transfers stay
                # at 112 partitions and spray across all 16 SDMA engines).
                step = WIN_COLS // parts
                for h in range(parts):
                    lo, hi = h * step, (h + 1) * step
                    nc.sync.dma_start(
                        slab[KD0:KDIM, lo:hi],
                        x_d[i, KD0:KDIM, wstart + lo:wstart + hi],
                    )
                    nc.sync.dma_start(
                        slab[0:KD0, lo:hi],
                        x_d[i, 0:KD0, wstart + lo:wstart + hi],
                    )
                return slab

            # finer-grained first load so the PE starts sooner; prefetch
            # two windows deep (bufs=3) to keep the DMA engines saturated.
            slabs = {0: load_window(0, 16), 1: load_window(1, 2)}
            for idx in range(len(windows)):
                if idx + 2 < len(windows):
                    slabs[idx + 2] = load_window(idx + 2, 2)
                slab = slabs.pop(idx)
                i, wi = windows[idx]
                wstart = wi * WIN_STRIDE
                blk0 = wi * WIN_BLOCKS
                thw = th_pool.tile([128, WIN_BLOCKS, J], f16)
                coff = 0
                for nb in CHUNKS:
                    mn = stage_pool.tile([128, 8, J], f16, tag="mn")
                    for q in range(0, nb, 4):
                        nq = min(4, nb - q)
                        # 4-bank PSUM tile: sub-block s at elem offset
                        # s*512 (bank-aligned) so one DVE reduce covers
                        # four blocks, amortizing reduce-op overhead.
                        ps = psum_pool.tile([128, 4, 512], f32)
                        for s in range(nq):
                            b = blk0 + coff + q + s
                            rel = b * BLK - wstart
                            for kh in range(3):
                                s0 = rel + kh * W
                                lhsT = (
                                    slab[:, s0:s0 + BLK]
                                    .rearrange("p (m j) -> p m j", m=128)
                                    [:, :, 0:1]
                                )
                                nc.tensor.matmul(
                                    ps[:, s, 0:NFREE],
                                    lhsT,
                                    w_t[:, kh * NFREE:(kh + 1) * NFREE],
                                    start=(kh == 0),
                                    stop=(kh == 2),
                                )
                        nc.vector.tensor_reduce(
                            mn[:, q:q + nq, :],
                            ps[:, 0:nq, 0:NFREE].rearrange(
                                "p s (j o) -> p s j o", o=C_OUT
                            ),
                            axis=mybir.AxisListType.X,
                            op=mybir.AluOpType.min,
                        )
                    nc.scalar.activation(
                        thw[:, coff:coff + nb, :], mn[:, 0:nb, :],
                        mybir.ActivationFunctionType.Tanh,
                    )
                    nc.scalar.activation(
                        thw[:, coff:coff + nb, :], thw[:, coff:coff + nb, :],
                        mybir.ActivationFunctionType.Tanh,
                    )
                    coff += nb
                # One batched store per window: 128 partition lines of
                # 510 f16 elems each (SWDGE, off the Sync FIFO). Split
                # 112+16: a >112-partition DMA lands on a single engine.
                nc.gpsimd.dma_start(y_d[i, wi, 0:112], thw[0:112])
                nc.gpsimd.dma_start(y_d[i, wi, 112:128], thw[112:128])
    nc.compile()
    return nc


_NC_CACHE = []


def _get_nc():
    if not _NC_CACHE:
        _NC_CACHE.append(_build_program())
    return _NC_CACHE[0]


def kernel(x, conv_weight, conv_bias, _trace=False):
    x = np.asarray(x, dtype=np.float32)
    conv_weight = np.asarray(conv_weight, dtype=np.float32)
    conv_bias = np.asarray(conv_bias, dtype=np.float32)
    n = x.shape[0]
    assert n == N_CORES * IMGS_PER_CORE

    x_aug, wmov = _prep_inputs(x, conv_weight, conv_bias)
    nc = _get_nc()
    in_maps = [
        {
            "x": np.ascontiguousarray(
                x_aug[c * IMGS_PER_CORE:(c + 1) * IMGS_PER_CORE]
            ),
            "w": wmov,
        }
        for c in range(N_CORES)
    ]
    res = run_bass_kernel_spmd(
        nc, in_maps, core_ids=list(range(N_CORES)), trace=_trace
    )
    # y: [4, 2, 128, 51, 5] per core -> flat position p = wi*32640 +
    # wb*640 + m*5 + j
    y = np.concatenate(
        [np.asarray(r["y"], dtype=np.float32) for r in res.results], axis=0
    )  # [32, 2, 128, 51, 5]
    y = y.transpose(0, 1, 3, 2, 4).reshape(n, 65280)
    y = y.reshape(n, 1, 255, 256)[:, :, :OH, :OW]
    out = np.ascontiguousarray(y)
    if _trace:
        kernel._last_result = res
    return out


# revision 20
# speedup vs baseline: 1.0314x; 1.0314x over previous
"""Trainium2 Bass kernel: conv2d(3x3,VALID) + bias -> min over C_out -> tanh(tanh).

Full-input contract: kernel(**inputs) takes the unsharded inputs
  x:           [32, 16, 256, 256] f32
  conv_weight: [64, 16, 3, 3]     f32
  conv_bias:   [64]               f32
and returns [32, 1, 254, 254] f32.

Strategy (data-parallel over batch, 4 images per core on 8 cores):
The conv is cast as matmuls over a "kw-shifted slab" in SBUF:
  slab[kw*16+c, t] = x[c, t+kw]   (kw in 0..6, flattened image cols t)
plus a ones row (partition 112) that carries the bias through the matmul.
For a block of 640 flat positions p = base + 5*m + j (m in 0..127, j in 0..4):
  out[m, (j,o)] = sum_kh  slab[:, base+kh*256+5m].T @ wmov[kh]
with wmov[kh][kw*16+c, j*64+o] = W[o,c,kh,kw-j] (0 when kw-j not in 0..2).
This yields PSUM [128 positions, 5 shifts, 64 channels]; the channel-min is
then a free-dim reduce_min on DVE, followed by tanh(tanh()) on ACT.
Computed on the full 256-wide rows; the host drops the 2 garbage columns/rows.

The slab is float8_e3m4: 1 byte/elem halves the HBM traffic that bounds the
fp16 version (~175 GB/s/core effective), and 4 mantissa bits keep the
x-quantization noise at ~1.6e-2 final rel err (vs the 2e-2 gate; e4m3's 3
mantissa bits measure 4.5e-2 — too coarse). The weights stay fp16 (they are
only ~217 KB, their dtype doesn't touch the DMA roofline) and the PE runs
mixed-dtype f8e3 x f16 matmuls, which TRN2 supports.

Outputs are buffered per window in SBUF [128, 51, 5] f16 and stored with one
SWDGE DMA of 128 contiguous 510-elem partition lines; the host de-permutes.
(Storing per-chunk in position order needs 8-byte descriptors — measured
~25us/engine of pure descriptor overhead across the 16 SDMA engines.)
"""

import sys
import types

import numpy as np
import ml_dtypes

# ---------------------------------------------------------------------------
# NTFF profile hook registration (the container's antenv stub lacks
# axon_hooks; registering it enables trace=True for profiling runs).
def _install_axon_hooks():
    try:
        import antenv.axon_hooks  # noqa: F401
        return
    except ImportError:
        pass
    try:
        import antenv
        from trn_agent_boot.trn_boot import _ntff_profile_via_ctypes
    except ImportError:
        return
    mod = types.ModuleType("antenv.axon_hooks")
    _hook = [None]
    mod.set_axon_ntff_profile_hook = lambda h: _hook.__setitem__(0, h)
    mod.get_axon_ntff_profile_hook = lambda: _hook[0]
    sys.modules["antenv.axon_hooks"] = mod
    antenv.axon_hooks = mod
    try:
        mod.set_axon_ntff_profile_hook(
            _ntff_profile_via_ctypes("/opt/axon/libaxon_pjrt.so")
        )
    except Exception:
        pass


_install_axon_hooks()

import concourse.bass as bass  # noqa: E402
import concourse.tile as tile  # noqa: E402
from concourse import bacc, mybir  # noqa: E402
from concourse.bass_utils import run_bass_kernel_spmd  # noqa: E402

N_CORES = 8
IMGS_PER_CORE = 4
C_IN, H, W = 16, 256, 256
C_OUT = 64
OH = OW = 254

J = 5                 # position shifts per matmul column group
WK = 7                # kw taps present in the contraction (0..6)
KDIM = WK * C_IN + 1  # 113 partitions: 7 kw-shifts x 16 ch + ones row
KD0 = WK * C_IN       # 112 data partitions
NFREE = J * C_OUT     # 320 moving columns
BLK = 128 * J         # 640 flat positions per block
IMG = H * W           # 65536
PAD_COLS = 66560      # 260 rows of 256 (2 extra conv rows + slack)
NBLOCKS = 102         # blocks 0..101 cover flat positions 0..65279

WIN_BLOCKS = 51       # two slab windows per image
WIN_STRIDE = WIN_BLOCKS * BLK      # 32640
WIN_COLS = 33792                   # window cols; max read rel offset 33153
CHUNKS = [8, 8, 8, 8, 8, 8, 3]     # blocks per output chunk inside a window


def _prep_inputs(x, conv_weight, conv_bias):
    """Host-side packing: slab-layout fp8 image tensor and fp16 weights.

    x7[i, kw*16+c, t] = x[i, c, t+kw] (kw in 0..6), row 112 = ones — exactly
    the SBUF slab layout, so device loads are single contiguous DMAs.
    """
    f8 = ml_dtypes.float8_e3m4
    n = x.shape[0]
    xf = x.reshape(n, C_IN, IMG).astype(f8)
    x7 = np.zeros((n, KDIM, PAD_COLS), dtype=f8)
    for kw in range(WK):
        lo = max(0, IMG - kw)
        x7[:, kw * C_IN:(kw + 1) * C_IN, :lo] = xf[:, :, kw:kw + lo]
    x7[:, KD0, :] = 1.0

    # wmov[kh][kw*16+c, j*64+o] = W[o, c, kh, kw-j] for kw-j in 0..2
    wmov = np.zeros((KDIM, 3, J, C_OUT), dtype=np.float32)
    for kh in range(3):
        for kw in range(WK):
            for j in range(J):
                kk = kw - j
                if 0 <= kk <= 2:
                    wmov[kw * C_IN:(kw + 1) * C_IN, kh, j, :] = (
                        conv_weight[:, :, kh, kk].T
                    )
    wmov[KD0, 0, :, :] = conv_bias[None, :]  # bias via ones row, kh=0 only
    # dram layout [113, 3*320] with col = kh*320 + j*64 + o
    wmov = wmov.reshape(KDIM, 3 * NFREE).astype(np.float16)
    return x7, wmov


def _build_program():
    nc = bacc.Bacc(
        "TRN2", target_bir_lowering=False, debug=False, num_devices=N_CORES
    )
    f8 = mybir.dt.float8e3
    f16 = mybir.dt.float16
    f32 = mybir.dt.float32

    x_d = nc.dram_tensor(
        "x", [IMGS_PER_CORE, KDIM, PAD_COLS], f8, kind="ExternalInput"
    )
    w_d = nc.dram_tensor("w", [KDIM, 3 * NFREE], f16, kind="ExternalInput")
    # y layout [img, window, m, window-block, j]; host de-permutes
    y_d = nc.dram_tensor(
        "y", [IMGS_PER_CORE, 2, 128, WIN_BLOCKS, J], f16,
        kind="ExternalOutput",
    )

    with tile.TileContext(nc) as tc:
        with (
            tc.tile_pool(name="wpool", bufs=1) as wpool,
            tc.tile_pool(name="slab", bufs=3) as slab_pool,
            tc.tile_pool(name="stage", bufs=4) as stage_pool,
            tc.tile_pool(name="thwin", bufs=2) as th_pool,
            tc.tile_pool(name="psum", bufs=2, space="PSUM") as psum_pool,
        ):
            w_t = wpool.tile([KDIM, 3 * NFREE], f16)
            # Split the weight load: a 113-partition transfer lands on a
            # single DMA engine (~16us serial at startup); 112+1 sprays.
            nc.sync.dma_start(w_t[0:KD0, :], w_d[0:KD0, :])
            nc.sync.dma_start(w_t[KD0:KDIM, :], w_d[KD0:KDIM, :])

            windows = [
                (i, wi) for i in range(IMGS_PER_CORE) for wi in range(2)
            ]

            def load_window(idx, parts):
                i, wi = windows[idx]
                wstart = wi * WIN_STRIDE
                slab = slab_pool.tile([KDIM, WIN_COLS], f8)
                # Ones row first (small, separate so the data transfers stay
                # at 112 partitions and spray across all 16 SDMA engines).
                step = WIN_COLS // parts
                for h in range(parts):
                    lo, hi = h * step, (h + 1) * step
                    nc.sync.dma_start(
                        slab[KD0:KDIM, lo:hi],
                        x_d[i, KD0:KDIM, wstart + lo:wstart + hi],
                    )
                    nc.sync.dma_start(
                        slab[0:KD0, lo:hi],
                        x_d[i, 0:KD0, wstart + lo:wstart + hi],
                    )
                return slab

            # finer-grained first load so the PE starts sooner; prefetch
            # two windows deep (bufs=3) to keep the DMA engines saturated.
            slabs = {0: load_window(0, 8), 1: load_window(1, 2)}
            for idx in range(len(windows)):
                if idx + 2 < len(windows):
                    slabs[idx + 2] = load_window(idx + 2, 2)
                slab = slabs.pop(idx)
                i, wi = windows[idx]
                wstart = wi * WIN_STRIDE
                blk0 = wi * WIN_BLOCKS
                thw = th_pool.tile([128, WIN_BLOCKS, J], f16)
                coff = 0
                for nb in CHUNKS:
                    mn = stage_pool.tile([128, 8, J], f16, tag="mn")
                    for q in range(0, nb, 4):
                        nq = min(4, nb - q)
                        # 4-bank PSUM tile: sub-block s at elem offset
                        # s*512 (bank-aligned) so one DVE reduce covers
                        # four blocks, amortizing reduce-op overhead.
                        ps = psum_pool.tile([128, 4, 512], f32)
                        for s in range(nq):
                            b = blk0 + coff + q + s
                            rel = b * BLK - wstart
                            for kh in range(3):
                                s0 = rel + kh * W
                                lhsT = (
                                    slab[:, s0:s0 + BLK]
                                    .rearrange("p (m j) -> p m j", m=128)
                                    [:, :, 0:1]
                                )
                                nc.tensor.matmul(
                                    ps[:, s, 0:NFREE],
                                    lhsT,
                                    w_t[:, kh * NFREE:(kh + 1) * NFREE],
                                    start=(kh == 0),
                                    stop=(kh == 2),
                                )
                        nc.vector.tensor_reduce(
                            mn[:, q:q + nq, :],
                            ps[:, 0:nq, 0:NFREE].rearrange(
                                "p s (j o) -> p s j o", o=C_OUT
                            ),
                            axis=mybir.AxisListType.X,
                            op=mybir.AluOpType.min,
                        )
                    nc.scalar.activation(
                        thw[:, coff:coff + nb, :], mn[:, 0:nb, :],
                        mybir.ActivationFunctionType.Tanh,
                    )
                    nc.scalar.activation(
                        thw[:, coff:coff + nb, :], thw[:, coff:coff + nb, :],
                        mybir.ActivationFunctionType.Tanh,
                    )
                    coff += nb
                # One batched store per window: 128 partition lines of
                # 510 f16 elems each (SWDGE, off the Sync FIFO). Split
                # 112+16: a >112-partition DMA lands on a single engine.
                nc.gpsimd.dma_start(y_d[i, wi, 0:112], thw[0:112])
                nc.gpsimd.dma_start(y_d[i, wi, 112:128], thw[112:128])
    nc.compile()
    return nc


_NC_CACHE = []


def _get_nc():
    if not _NC_CACHE:
        _NC_CACHE.append(_build_program())
    return _NC_CACHE[0]


def kernel(x, conv_weight, conv_bias, _trace=False):
    x = np.asarray(x, dtype=np.float32)
    conv_weight = np.asarray(conv_weight, dtype=np.float32)
    conv_bias = np.asarray(conv_bias, dtype=np.float32)
    n = x.shape[0]
    assert n == N_CORES * IMGS_PER_CORE

    x_aug, wmov = _prep_inputs(x, conv_weight, conv_bias)
    nc = _get_nc()
    in_maps = [
        {
            "x": np.ascontiguousarray(
                x_aug[c * IMGS_PER_CORE:(c + 1) * IMGS_PER_CORE]
            ),
            "w": wmov,
        }
        for c in range(N_CORES)
    ]
    res = run_bass_kernel_spmd(
        nc, in_maps, core_ids=list(range(N_CORES)), trace=_trace
    )
    # y: [4, 2, 128, 51, 5] per core -> flat position p = wi*32640 +
    # wb*640 + m*5 + j
    y = np.concatenate(
        [np.asarray(r["y"], dtype=np.float32) for r in res.results], axis=0
    )  # [32, 2, 128, 51, 5]
    y = y.transpose(0, 1, 3, 2, 4).reshape(n, 65280)
    y = y.reshape(n, 1, 255, 256)[:, :, :OH, :OW]
    out = np.ascontiguousarray(y)
    if _trace:
        kernel._last_result = res
    return out


# revision 22
# speedup vs baseline: 1.0968x; 1.0634x over previous
"""Trainium2 Bass kernel: conv2d(3x3,VALID) + bias -> min over C_out -> tanh(tanh).

Full-input contract: kernel(**inputs) takes the unsharded inputs
  x:           [32, 16, 256, 256] f32
  conv_weight: [64, 16, 3, 3]     f32
  conv_bias:   [64]               f32
and returns [32, 1, 254, 254] f32.

Strategy (data-parallel over batch, 4 images per core on 8 cores):
The conv is cast as matmuls over a "kw-shifted slab" in SBUF:
  slab[kw*16+c, t] = x[c, t+kw]   (kw in 0..6, flattened image cols t)
plus a ones row (partition 112) that carries the bias through the matmul.
For a block of 640 flat positions p = base + 5*m + j (m in 0..127, j in 0..4):
  out[m, (j,o)] = sum_kh  slab[:, base+kh*256+5m].T @ wmov[kh]
with wmov[kh][kw*16+c, j*64+o] = W[o,c,kh,kw-j] (0 when kw-j not in 0..2).
This yields PSUM [128 positions, 5 shifts, 64 channels]; the channel-min is
then a free-dim reduce_min on DVE, followed by tanh(tanh()) on ACT.
Computed on the full 256-wide rows; the host drops the 2 garbage columns/rows.

The slab is float8_e3m4: 1 byte/elem halves the HBM traffic that bounds the
fp16 version (~175 GB/s/core effective), and 4 mantissa bits keep the
x-quantization noise at ~1.6e-2 final rel err (vs the 2e-2 gate; e4m3's 3
mantissa bits measure 4.5e-2 — too coarse). The weights stay fp16 (they are
only ~217 KB, their dtype doesn't touch the DMA roofline) and the PE runs
mixed-dtype f8e3 x f16 matmuls, which TRN2 supports.

Outputs are buffered per window in SBUF [128, 51, 5] f16 and stored with one
SWDGE DMA of 128 contiguous 510-elem partition lines; the host de-permutes.
(Storing per-chunk in position order needs 8-byte descriptors — measured
~25us/engine of pure descriptor overhead across the 16 SDMA engines.)
"""

import sys
import types

import numpy as np
import ml_dtypes

# ---------------------------------------------------------------------------
# NTFF profile hook registration (the container's antenv stub lacks
# axon_hooks; registering it enables trace=True for profiling runs).
def _install_axon_hooks():
    try:
        import antenv.axon_hooks  # noqa: F401
        return
    except ImportError:
        pass
    try:
        import antenv
        from trn_agent_boot.trn_boot import _ntff_profile_via_ctypes
    except ImportError:
        return
    mod = types.ModuleType("antenv.axon_hooks")
    _hook = [None]
    mod.set_axon_ntff_profile_hook = lambda h: _hook.__setitem__(0, h)
    mod.get_axon_ntff_profile_hook = lambda: _hook[0]
    sys.modules["antenv.axon_hooks"] = mod
    antenv.axon_hooks = mod
    try:
        mod.set_axon_ntff_profile_hook(
            _ntff_profile_via_ctypes("/opt/axon/libaxon_pjrt.so")
        )
    except Exception:
        pass


_install_axon_hooks()

import concourse.bass as bass  # noqa: E402
import concourse.tile as tile  # noqa: E402
from concourse import bacc, mybir  # noqa: E402
from concourse.bass_utils import run_bass_kernel_spmd  # noqa: E402

N_CORES = 8
IMGS_PER_CORE = 4
C_IN, H, W = 16, 256, 256
C_OUT = 64
OH = OW = 254

J = 5                 # position shifts per matmul column group
WK = 7                # kw taps present in the contraction (0..6)
KDIM = WK * C_IN + 1  # 113 partitions: 7 kw-shifts x 16 ch + ones row
KD0 = WK * C_IN       # 112 data partitions
NFREE = J * C_OUT     # 320 moving columns
BLK = 128 * J         # 640 flat positions per block
IMG = H * W           # 65536
PAD_COLS = 66560      # 260 rows of 256 (2 extra conv rows + slack)
NBLOCKS = 102         # blocks 0..101 cover flat positions 0..65279

WIN_BLOCKS = 51       # two slab windows per image
WIN_STRIDE = WIN_BLOCKS * BLK      # 32640
WIN_COLS = 33792                   # window cols; max read rel offset 33153
CHUNKS = [8, 8, 8, 8, 8, 8, 3]     # blocks per output chunk inside a window


def _prep_inputs(x, conv_weight, conv_bias):
    """Host-side packing: slab-layout fp8 image tensor and fp16 weights.

    x7[i, kw*16+c, t] = x[i, c, t+kw] (kw in 0..6), row 112 = ones — exactly
    the SBUF slab layout, so device loads are single contiguous DMAs.
    """
    f8 = ml_dtypes.float8_e3m4
    n = x.shape[0]
    xf = x.reshape(n, C_IN, IMG).astype(f8)
    x7 = np.zeros((n, KDIM, PAD_COLS), dtype=f8)
    for kw in range(WK):
        lo = max(0, IMG - kw)
        x7[:, kw * C_IN:(kw + 1) * C_IN, :lo] = xf[:, :, kw:kw + lo]
    x7[:, KD0, :] = 1.0

    # wmov[kh][kw*16+c, j*64+o] = W[o, c, kh, kw-j] for kw-j in 0..2
    wmov = np.zeros((KDIM, 3, J, C_OUT), dtype=np.float32)
    for kh in range(3):
        for kw in range(WK):
            for j in range(J):
                kk = kw - j
                if 0 <= kk <= 2:
                    wmov[kw * C_IN:(kw + 1) * C_IN, kh, j, :] = (
                        conv_weight[:, :, kh, kk].T
                    )
    wmov[KD0, 0, :, :] = conv_bias[None, :]  # bias via ones row, kh=0 only
    # dram layout [113, 3*320] with col = kh*320 + j*64 + o
    wmov = wmov.reshape(KDIM, 3 * NFREE).astype(np.float16)
    return x7, wmov


def _build_program():
    nc = bacc.Bacc(
        "TRN2", target_bir_lowering=False, debug=False, num_devices=N_CORES
    )
    f8 = mybir.dt.float8e3
    f16 = mybir.dt.float16
    f32 = mybir.dt.float32

    x_d = nc.dram_tensor(
        "x", [IMGS_PER_CORE, KDIM, PAD_COLS], f8, kind="ExternalInput"
    )
    w_d = nc.dram_tensor("w", [KDIM, 3 * NFREE], f16, kind="ExternalInput")
    # y layout [img, window, m, window-block, j]; host de-permutes
    y_d = nc.dram_tensor(
        "y", [IMGS_PER_CORE, 2, 128, WIN_BLOCKS, J], f16,
        kind="ExternalOutput",
    )

    with tile.TileContext(nc) as tc:
        with (
            tc.tile_pool(name="wpool", bufs=1) as wpool,
            tc.tile_pool(name="slab", bufs=3) as slab_pool,
            tc.tile_pool(name="stage", bufs=4) as stage_pool,
            tc.tile_pool(name="thwin", bufs=2) as th_pool,
            tc.tile_pool(name="psum", bufs=4, space="PSUM") as psum_pool,
        ):
            w_t = wpool.tile([KDIM, 3 * NFREE], f16)
            # Split the weight load: a 113-partition transfer lands on a
            # single DMA engine (~16us serial at startup); 112+1 sprays.
            nc.sync.dma_start(w_t[0:KD0, :], w_d[0:KD0, :])
            nc.sync.dma_start(w_t[KD0:KDIM, :], w_d[KD0:KDIM, :])

            windows = [
                (i, wi) for i in range(IMGS_PER_CORE) for wi in range(2)
            ]

            def load_window(idx, parts):
                i, wi = windows[idx]
                wstart = wi * WIN_STRIDE
                slab = slab_pool.tile([KDIM, WIN_COLS], f8)
                # Ones row first (small, separate so the data transfers stay
                # at 112 partitions and spray across all 16 SDMA engines).
                step = WIN_COLS // parts
                for h in range(parts):
                    lo, hi = h * step, (h + 1) * step
                    nc.sync.dma_start(
                        slab[KD0:KDIM, lo:hi],
                        x_d[i, KD0:KDIM, wstart + lo:wstart + hi],
                    )
                    nc.sync.dma_start(
                        slab[0:KD0, lo:hi],
                        x_d[i, 0:KD0, wstart + lo:wstart + hi],
                    )
                return slab

            # finer-grained first load so the PE starts sooner; prefetch
            # two windows deep (bufs=3) to keep the DMA engines saturated.
            slabs = {0: load_window(0, 8), 1: load_window(1, 2)}
            for idx in range(len(windows)):
                if idx + 2 < len(windows):
                    slabs[idx + 2] = load_window(idx + 2, 2)
                slab = slabs.pop(idx)
                i, wi = windows[idx]
                wstart = wi * WIN_STRIDE
                blk0 = wi * WIN_BLOCKS
                thw = th_pool.tile([128, WIN_BLOCKS, J], f16)
                coff = 0
                for nb in CHUNKS:
                    mn = stage_pool.tile([128, 8, J], f16, tag="mn")
                    for q in range(0, nb, 2):
                        nq = min(2, nb - q)
                        # 2-bank PSUM tile: sub-block s at elem offset
                        # s*512 (bank-aligned) so one DVE reduce covers
                        # two blocks; 4 bufs keep more chains in flight.
                        ps = psum_pool.tile([128, 2, 512], f32)
                        for s in range(nq):
                            b = blk0 + coff + q + s
                            rel = b * BLK - wstart
                            for kh in range(3):
                                s0 = rel + kh * W
                                lhsT = (
                                    slab[:, s0:s0 + BLK]
                                    .rearrange("p (m j) -> p m j", m=128)
                                    [:, :, 0:1]
                                )
                                nc.tensor.matmul(
                                    ps[:, s, 0:NFREE],
                                    lhsT,
                                    w_t[:, kh * NFREE:(kh + 1) * NFREE],
                                    start=(kh == 0),
                                    stop=(kh == 2),
                                )
                        nc.vector.tensor_reduce(
                            mn[:, q:q + nq, :],
                            ps[:, 0:nq, 0:NFREE].rearrange(
                                "p s (j o) -> p s j o", o=C_OUT
                            ),
                            axis=mybir.AxisListType.X,
                            op=mybir.AluOpType.min,
                        )
                    nc.scalar.activation(
                        thw[:, coff:coff + nb, :], mn[:, 0:nb, :],
                        mybir.ActivationFunctionType.Tanh,
                    )
                    nc.scalar.activation(
                        thw[:, coff:coff + nb, :], thw[:, coff:coff + nb, :],
                        mybir.ActivationFunctionType.Tanh,
                    )
                    coff += nb
                # One batched store per window: 128 partition lines of
                # 510 f16 elems each (SWDGE, off the Sync FIFO). Split
                # 112+16: a >112-partition DMA lands on a single engine.
                nc.gpsimd.dma_start(y_d[i, wi, 0:112], thw[0:112])
                nc.gpsimd.dma_start(y_d[i, wi, 112:128], thw[112:128])
    nc.compile()
    return nc


_NC_CACHE = []


def _get_nc():
    if not _NC_CACHE:
        _NC_CACHE.append(_build_program())
    return _NC_CACHE[0]


def kernel(x, conv_weight, conv_bias, _trace=False):
    x = np.asarray(x, dtype=np.float32)
    conv_weight = np.asarray(conv_weight, dtype=np.float32)
    conv_bias = np.asarray(conv_bias, dtype=np.float32)
    n = x.shape[0]
    assert n == N_CORES * IMGS_PER_CORE

    x_aug, wmov = _prep_inputs(x, conv_weight, conv_bias)
    nc = _get_nc()
    in_maps = [
        {
            "x": np.ascontiguousarray(
                x_aug[c * IMGS_PER_CORE:(c + 1) * IMGS_PER_CORE]
            ),
            "w": wmov,
        }
        for c in range(N_CORES)
    ]
    res = run_bass_kernel_spmd(
        nc, in_maps, core_ids=list(range(N_CORES)), trace=_trace
    )
    # y: [4, 2, 128, 51, 5] per core -> flat position p = wi*32640 +
    # wb*640 + m*5 + j
    y = np.concatenate(
        [np.asarray(r["y"], dtype=np.float32) for r in res.results], axis=0
    )  # [32, 2, 128, 51, 5]
    y = y.transpose(0, 1, 3, 2, 4).reshape(n, 65280)
    y = y.reshape(n, 1, 255, 256)[:, :, :OH, :OW]
    out = np.ascontiguousarray(y)
    if _trace:
        kernel._last_result = res
    return out


# revision 24
# speedup vs baseline: 1.0993x; 1.0023x over previous
"""Trainium2 Bass kernel: conv2d(3x3,VALID) + bias -> min over C_out -> tanh(tanh).

Full-input contract: kernel(**inputs) takes the unsharded inputs
  x:           [32, 16, 256, 256] f32
  conv_weight: [64, 16, 3, 3]     f32
  conv_bias:   [64]               f32
and returns [32, 1, 254, 254] f32.

Strategy (data-parallel over batch, 4 images per core on 8 cores):
The conv is cast as matmuls over a "kw-shifted slab" in SBUF:
  slab[kw*16+c, t] = x[c, t+kw]   (kw in 0..6, flattened image cols t)
plus a ones row (partition 112) that carries the bias through the matmul.
For a block of 640 flat positions p = base + 5*m + j (m in 0..127, j in 0..4):
  out[m, (j,o)] = sum_kh  slab[:, base+kh*256+5m].T @ wmov[kh]
with wmov[kh][kw*16+c, j*64+o] = W[o,c,kh,kw-j] (0 when kw-j not in 0..2).
This yields PSUM [128 positions, 5 shifts, 64 channels]; the channel-min is
then a free-dim reduce_min on DVE, followed by tanh(tanh()) on ACT.
Computed on the full 256-wide rows; the host drops the 2 garbage columns/rows.

The slab is float8_e3m4: 1 byte/elem halves the HBM traffic that bounds the
fp16 version (~175 GB/s/core effective), and 4 mantissa bits keep the
x-quantization noise at ~1.6e-2 final rel err (vs the 2e-2 gate; e4m3's 3
mantissa bits measure 4.5e-2 — too coarse). The weights stay fp16 (they are
only ~217 KB, their dtype doesn't touch the DMA roofline) and the PE runs
mixed-dtype f8e3 x f16 matmuls, which TRN2 supports.

Outputs are buffered per window in SBUF [128, 51, 5] f16 and stored with one
SWDGE DMA of 128 contiguous 510-elem partition lines; the host de-permutes.
(Storing per-chunk in position order needs 8-byte descriptors — measured
~25us/engine of pure descriptor overhead across the 16 SDMA engines.)
"""

import sys
import types

import numpy as np
import ml_dtypes

# ---------------------------------------------------------------------------
# NTFF profile hook registration (the container's antenv stub lacks
# axon_hooks; registering it enables trace=True for profiling runs).
def _install_axon_hooks():
    try:
        import antenv.axon_hooks  # noqa: F401
        return
    except ImportError:
        pass
    try:
        import antenv
        from trn_agent_boot.trn_boot import _ntff_profile_via_ctypes
    except ImportError:
        return
    mod = types.ModuleType("antenv.axon_hooks")
    _hook = [None]
    mod.set_axon_ntff_profile_hook = lambda h: _hook.__setitem__(0, h)
    mod.get_axon_ntff_profile_hook = lambda: _hook[0]
    sys.modules["antenv.axon_hooks"] = mod
    antenv.axon_hooks = mod
    try:
        mod.set_axon_ntff_profile_hook(
            _ntff_profile_via_ctypes("/opt/axon/libaxon_pjrt.so")
        )
    except Exception:
        pass


_install_axon_hooks()

import concourse.bass as bass  # noqa: E402
import concourse.tile as tile  # noqa: E402
from concourse import bacc, mybir  # noqa: E402
from concourse.bass_utils import run_bass_kernel_spmd  # noqa: E402

N_CORES = 8
IMGS_PER_CORE = 4
C_IN, H, W = 16, 256, 256
C_OUT = 64
OH = OW = 254

J = 5                 # position shifts per matmul column group
WK = 7                # kw taps present in the contraction (0..6)
KDIM = WK * C_IN + 1  # 113 partitions: 7 kw-shifts x 16 ch + ones row
KD0 = WK * C_IN       # 112 data partitions
NFREE = J * C_OUT     # 320 moving columns
BLK = 128 * J         # 640 flat positions per block
IMG = H * W           # 65536
PAD_COLS = 66560      # 260 rows of 256 (2 extra conv rows + slack)
NBLOCKS = 102         # blocks 0..101 cover flat positions 0..65279

WIN_BLOCKS = 51       # two slab windows per image
WIN_STRIDE = WIN_BLOCKS * BLK      # 32640
WIN_COLS = 33792                   # window cols; max read rel offset 33153
CHUNKS = [8, 8, 8, 8, 8, 8, 3]     # blocks per output chunk inside a window


def _prep_inputs(x, conv_weight, conv_bias):
    """Host-side packing: slab-layout fp8 image tensor and fp16 weights.

    x7[i, kw*16+c, t] = x[i, c, t+kw] (kw in 0..6), row 112 = ones — exactly
    the SBUF slab layout, so device loads are single contiguous DMAs.
    """
    f8 = ml_dtypes.float8_e3m4
    n = x.shape[0]
    xf = x.reshape(n, C_IN, IMG).astype(f8)
    x7 = np.zeros((n, KDIM, PAD_COLS), dtype=f8)
    for kw in range(WK):
        lo = max(0, IMG - kw)
        x7[:, kw * C_IN:(kw + 1) * C_IN, :lo] = xf[:, :, kw:kw + lo]
    x7[:, KD0, :] = 1.0

    # wmov[kh][kw*16+c, j*64+o] = W[o, c, kh, kw-j] for kw-j in 0..2
    wmov = np.zeros((KDIM, 3, J, C_OUT), dtype=np.float32)
    for kh in range(3):
        for kw in range(WK):
            for j in range(J):
                kk = kw - j
                if 0 <= kk <= 2:
                    wmov[kw * C_IN:(kw + 1) * C_IN, kh, j, :] = (
                        conv_weight[:, :, kh, kk].T
                    )
    wmov[KD0, 0, :, :] = conv_bias[None, :]  # bias via ones row, kh=0 only
    # dram layout [113, 3*320] with col = kh*320 + j*64 + o
    wmov = wmov.reshape(KDIM, 3 * NFREE).astype(np.float16)
    return x7, wmov


def _build_program():
    nc = bacc.Bacc(
        "TRN2", target_bir_lowering=False, debug=False, num_devices=N_CORES
    )
    f8 = mybir.dt.float8e3
    f16 = mybir.dt.float16
    f32 = mybir.dt.float32

    x_d = nc.dram_tensor(
        "x", [IMGS_PER_CORE, KDIM, PAD_COLS], f8, kind="ExternalInput"
    )
    w_d = nc.dram_tensor("w", [KDIM, 3 * NFREE], f16, kind="ExternalInput")
    # y layout [img, window, m, window-block, j]; host de-permutes
    y_d = nc.dram_tensor(
        "y", [IMGS_PER_CORE, 2, 128, WIN_BLOCKS, J], f16,
        kind="ExternalOutput",
    )

    with tile.TileContext(nc) as tc:
        with (
            tc.tile_pool(name="wpool", bufs=1) as wpool,
            tc.tile_pool(name="slab", bufs=3) as slab_pool,
            tc.tile_pool(name="stage", bufs=4) as stage_pool,
            tc.tile_pool(name="thwin", bufs=2) as th_pool,
            tc.tile_pool(name="psum", bufs=4, space="PSUM") as psum_pool,
        ):
            w_t = wpool.tile([KDIM, 3 * NFREE], f16)
            # Split the weight load: a 113-partition transfer lands on a
            # single DMA engine (~16us serial at startup); 112+1 sprays.
            nc.sync.dma_start(w_t[0:KD0, :], w_d[0:KD0, :])
            nc.sync.dma_start(w_t[KD0:KDIM, :], w_d[KD0:KDIM, :])

            windows = [
                (i, wi) for i in range(IMGS_PER_CORE) for wi in range(2)
            ]

            def load_window(idx, bounds):
                i, wi = windows[idx]
                wstart = wi * WIN_STRIDE
                slab = slab_pool.tile([KDIM, WIN_COLS], f8)
                # Ones row first (small, separate so the data transfers stay
                # at 112 partitions and spray across all 16 SDMA engines).
                for lo, hi in zip(bounds[:-1], bounds[1:]):
                    nc.sync.dma_start(
                        slab[KD0:KDIM, lo:hi],
                        x_d[i, KD0:KDIM, wstart + lo:wstart + hi],
                    )
                    nc.sync.dma_start(
                        slab[0:KD0, lo:hi],
                        x_d[i, 0:KD0, wstart + lo:wstart + hi],
                    )
                return slab

            # Uneven first load: a tiny part 0 lets the PE start during the
            # NEFF preamble (sync-queue issue costs ~0.7us per dma_start, so
            # few-but-uneven beats many-fine parts). Prefetch two windows
            # deep (bufs=3) to keep the DMA engines saturated.
            HALF = [0, WIN_COLS // 2, WIN_COLS]
            slabs = {
                0: load_window(0, [0, 2112, 8448, 16896, 25344, WIN_COLS]),
                1: load_window(1, HALF),
            }
            for idx in range(len(windows)):
                if idx + 2 < len(windows):
                    slabs[idx + 2] = load_window(idx + 2, HALF)
                slab = slabs.pop(idx)
                i, wi = windows[idx]
                wstart = wi * WIN_STRIDE
                blk0 = wi * WIN_BLOCKS
                thw = th_pool.tile([128, WIN_BLOCKS, J], f16)
                coff = 0
                for nb in CHUNKS:
                    mn = stage_pool.tile([128, 8, J], f16, tag="mn")
                    for q in range(0, nb, 2):
                        nq = min(2, nb - q)
                        # 2-bank PSUM tile: sub-block s at elem offset
                        # s*512 (bank-aligned) so one DVE reduce covers
                        # two blocks; 4 bufs keep more chains in flight.
                        ps = psum_pool.tile([128, 2, 512], f32)
                        for s in range(nq):
                            b = blk0 + coff + q + s
                            rel = b * BLK - wstart
                            for kh in range(3):
                                s0 = rel + kh * W
                                lhsT = (
                                    slab[:, s0:s0 + BLK]
                                    .rearrange("p (m j) -> p m j", m=128)
                                    [:, :, 0:1]
                                )
                                nc.tensor.matmul(
                                    ps[:, s, 0:NFREE],
                                    lhsT,
                                    w_t[:, kh * NFREE:(kh + 1) * NFREE],
                                    start=(kh == 0),
                                    stop=(kh == 2),
                                )
                        nc.vector.tensor_reduce(
                            mn[:, q:q + nq, :],
                            ps[:, 0:nq, 0:NFREE].rearrange(
                                "p s (j o) -> p s j o", o=C_OUT
                            ),
                            axis=mybir.AxisListType.X,
                            op=mybir.AluOpType.min,
                        )
                    nc.scalar.activation(
                        thw[:, coff:coff + nb, :], mn[:, 0:nb, :],
                        mybir.ActivationFunctionType.Tanh,
                    )
                    nc.scalar.activation(
                        thw[:, coff:coff + nb, :], thw[:, coff:coff + nb, :],
                        mybir.ActivationFunctionType.Tanh,
                    )
                    coff += nb
                # One batched store per window: 128 partition lines of
                # 510 f16 elems each (SWDGE, off the Sync FIFO). Split
                # 112+16: a >112-partition DMA lands on a single engine.
                nc.gpsimd.dma_start(y_d[i, wi, 0:112], thw[0:112])
                nc.gpsimd.dma_start(y_d[i, wi, 112:128], thw[112:128])
    nc.compile()
    return nc


_NC_CACHE = []


def _get_nc():
    if not _NC_CACHE:
        _NC_CACHE.append(_build_program())
    return _NC_CACHE[0]


def kernel(x, conv_weight, conv_bias, _trace=False):
    x = np.asarray(x, dtype=np.float32)
    conv_weight = np.asarray(conv_weight, dtype=np.float32)
    conv_bias = np.asarray(conv_bias, dtype=np.float32)
    n = x.shape[0]
    assert n == N_CORES * IMGS_PER_CORE

    x_aug, wmov = _prep_inputs(x, conv_weight, conv_bias)
    nc = _get_nc()
    in_maps = [
        {
            "x": np.ascontiguousarray(
                x_aug[c * IMGS_PER_CORE:(c + 1) * IMGS_PER_CORE]
            ),
            "w": wmov,
        }
        for c in range(N_CORES)
    ]
    res = run_bass_kernel_spmd(
        nc, in_maps, core_ids=list(range(N_CORES)), trace=_trace
    )
    # y: [4, 2, 128, 51, 5] per core -> flat position p = wi*32640 +
    # wb*640 + m*5 + j
    y = np.concatenate(
        [np.asarray(r["y"], dtype=np.float32) for r in res.results], axis=0
    )  # [32, 2, 128, 51, 5]
    y = y.transpose(0, 1, 3, 2, 4).reshape(n, 65280)
    y = y.reshape(n, 1, 255, 256)[:, :, :OH, :OW]
    out = np.ascontiguousarray(y)
    if _trace:
        kernel._last_result = res
    return out
